# revision 20
# baseline (speedup 1.0000x reference)
"""Trainium2 Bass kernel for nn_CA3RecurrentMatrix (scatter_memory).

Math: the reference's Ben-Israel-Cohen pseudoinverse iteration collapses
algebraically.  With pinv_0 = alpha*A^T, every iterate is pinv_n = p_n(G) A^T
with G = A^T A (C x C), and on eigenvalues g of G the output polynomial is
u_8(g)*g = 1 - (1 - alpha*g)^256 = 256*alpha*g - C(256,2)*(alpha*g)^2 + ...
Because alpha <= 5e-4/||A||_F^2, alpha*g_max ~ 7e-7, so even the QUADRATIC
term is <1e-4 relative (verified numerically: dropping it gives 5.7e-5 max
rel err).  Hence, to well within the 2e-2 gate:

    out = (256*alpha) * query @ (A^T A)

Distribution over 8 cores, all bf16 compute (validated 4.2e-3 max rel err):
core i holds W_i = A[:, R_i] (bf16) and computes G row-block
G[R_i, :] = W_i^T A in four 512-column chunks; each chunk is AllGathered
(bf16, ~2MB) as soon as it is ready, and the big GEMM out_i = Q_i @ G
consumes gathered column-chunks as they land.  The partial Frobenius sum
sum(W_i^2) rides as one extra payload row in the FIRST AllGather (f32
bit-split into two bf16 slots), so there is no separate AllReduce on the
serial CC ring; the scale s = 256*min(exp(ls),5e-4)/(fro2+1e-8) is applied
during the PSUM->SBUF eviction of the output tiles.  Queue discipline
(each engine DMA queue is strict FIFO, so placement avoids head-of-line
blocking): sync = W + A-stream evens + out stores; scalar = exp(ls) +
A-stream odds + QT + gathered-G loads; gpsimd = gin stores + all four
AllGather triggers pre-queued back-to-back (nothing between them waits on
a prior AG, so the meshes chain with ~2us gaps instead of ~30us
rendezvous stalls).  The first collective mesh cannot begin before a
fixed ~84us firmware-init wall regardless of trigger time, which sets the
floor: ~84 (wall) + ~28 (AG0) + ~135 (tensor-bound GEMM3 at ~259ns per
512-row matmul) ~= 250-275us.
"""
import sys, os, types

sys.path.insert(0, "/opt/trn_rl_repo")

import numpy as np

B, C, K = 8192, 2048, 4096
NCORES = 8
CB = C // NCORES     # 256 G-row block per core
BB = B // NCORES     # 1024 query rows per core
KT = K // 128        # 32 k-tiles over K
CT = C // 128        # 16 c-tiles over C
CW = 512             # column-chunk width
NCH = C // CW        # 4 column chunks
ALPHA_CLAMP = 5e-4
C1 = 256.0

_CACHE = {}


def _install_ntff_shim():
    """Make trace=True work under axon (antenv.axon_hooks is absent here)."""
    if "antenv.axon_hooks" in sys.modules:
        return
    try:
        import antenv
    except ImportError:
        return
    mod = types.ModuleType("antenv.axon_hooks")
    state = {"hook": None, "resolved": False}

    def set_axon_ntff_profile_hook(hook):
        state["hook"], state["resolved"] = hook, True

    def get_axon_ntff_profile_hook():
        if not state["resolved"]:
            state["resolved"] = True
            try:
                if "/root/.axon_site" not in sys.path:
                    sys.path.insert(0, "/root/.axon_site")
                from trn_agent_boot.trn_boot import _ntff_profile_via_ctypes
                state["hook"] = _ntff_profile_via_ctypes("/opt/axon/libaxon_pjrt.so")
            except Exception:
                state["hook"] = None
        return state["hook"]

    mod.set_axon_ntff_profile_hook = set_axon_ntff_profile_hook
    mod.get_axon_ntff_profile_hook = get_axon_ntff_profile_hook
    sys.modules["antenv.axon_hooks"] = mod
    antenv.axon_hooks = mod


def build_nc():
    import concourse.bacc as bacc
    import concourse.mybir as mybir
    from concourse import tile

    f32 = mybir.dt.float32
    bf16 = mybir.dt.bfloat16
    RG = [list(range(NCORES))]

    nc = bacc.Bacc("TRN2", target_bir_lowering=False, debug=False,
                   num_devices=NCORES)
    a_d = nc.dram_tensor("a", (K, C), bf16, kind="ExternalInput")
    # host pre-swizzled: [128, KT*CB] so rows are 16KB contiguous
    w_d = nc.dram_tensor("w", (128, KT * CB), bf16, kind="ExternalInput")
    qt_d = nc.dram_tensor("qt", (C, BB), bf16, kind="ExternalInput")
    ls_d = nc.dram_tensor("ls", (128, 1), f32, kind="ExternalInput")
    out_d = nc.dram_tensor("out", (BB, C), f32, kind="ExternalOutput")

    with tile.TileContext(nc) as tc:
        with tc.tile_pool(name="sbuf", bufs=1) as pool, \
             tc.tile_pool(name="psum", bufs=1, space="PSUM") as psum, \
             tc.tile_pool(name="dram", bufs=1, space="DRAM") as dram:
            # chunk 0 carries one extra payload row per rank: the partial
            # Frobenius sum (f32 bit-split into two bf16 slots)
            gin0 = dram.tile([CB + 1, CW], bf16, name="gin0")
            gout0 = dram.tile([(CB + 1) * NCORES, CW], bf16,
                              addr_space="Shared", name="gout0")
            gin = [gin0] + [dram.tile([CB, CW], bf16, name=f"gin{n}")
                            for n in range(1, NCH)]
            gout = [gout0] + [dram.tile([C, CW], bf16, addr_space="Shared",
                                        name=f"gout{n}")
                              for n in range(1, NCH)]

            ls_sb = pool.tile([128, 1], f32, tag="ls")
            nc.gpsimd.dma_start(ls_sb[:], ls_d.ap()[:, :])
            # exp(ls) early on the scalar queue (before QT) so the alpha
            # chain never waits on scalar-queue HOL
            ex = pool.tile([128, 1], f32, tag="ex")
            nc.scalar.activation(ex[:], ls_sb[:],
                                 mybir.ActivationFunctionType.Exp)

            # W resident: [128, KT*CB], k-tile k at cols [k*CB:(k+1)*CB].
            # 4 DMAs so GEMM1 k-tile 0 isn't gated on the whole 2MB.
            wt = pool.tile([128, KT * CB], bf16, tag="wt")
            for j in range(4):
                sl = slice(j * 8 * CB, (j + 1) * 8 * CB)
                nc.sync.dma_start(wt[:, sl], w_d.ap()[:, sl])

            # PE warm-up: garbage matmuls on the first W quarter to lift the
            # HAM clock gate while the A-stream lands (never read back)
            psw = psum.tile([128, 512], f32, tag="pb3", name="psw")
            for _ in range(8):
                nc.tensor.matmul(psw[:], wt[:, 0:128], wt[:, 0:512],
                                 start=True, stop=True)

            # ---- partial fro2 = sum(W_i^2) on DVE ----
            with nc.named_scope("fro2"):
                dp = pool.tile([128, 4], f32, tag="dp")
                for j in range(4):
                    sl = slice(j * 8 * CB, (j + 1) * 8 * CB)
                    sq = pool.tile([128, 8 * CB], f32, tag="sq", bufs=1)
                    nc.vector.tensor_mul(sq[:], wt[:, sl], wt[:, sl])
                    nc.vector.reduce_sum(dp[:, j:j + 1], sq[:],
                                         axis=mybir.AxisListType.X)
                dx = pool.tile([128, 1], f32, tag="dx")
                nc.vector.reduce_sum(dx[:], dp[:], axis=mybir.AxisListType.X)
                fro_p = pool.tile([1, 1], f32, tag="frop")
                nc.gpsimd.tensor_reduce(fro_p[:], dx[:],
                                        op=mybir.AluOpType.add,
                                        axis=mybir.AxisListType.C)
                # ride along in AllGather chunk 0 (bit-exact f32 in 2 bf16)
                # NB: on gpsimd, not scalar — scalar would HOL-block the
                # A-odd stream behind fro_p's dependency
                nc.gpsimd.dma_start(gin0[CB:CB + 1, 0:2].bitcast(f32),
                                    fro_p[:])

            # ---- GEMM1: two 1024-wide passes over A (2KB DMA rows, split
            #      across sync+scalar); evict per-512 chunk and AllGather
            #      each chunk as soon as its pass completes ----
            with nc.named_scope("gemm1"):
                for p in range(2):
                    psg = [psum.tile([128, 1024], f32, tag=f"pa{m}",
                                     name=f"psg{p}_{m}") for m in range(2)]
                    for k in range(KT):
                        ak = pool.tile([128, 1024], bf16, tag="ak", bufs=8)
                        eng = nc.sync if k % 2 == 0 else nc.scalar
                        eng.dma_start(ak[:],
                                      a_d.ap()[k * 128:(k + 1) * 128,
                                               p * 1024:(p + 1) * 1024])
                        for m in range(2):
                            for h in range(2):
                                nc.tensor.matmul(
                                    psg[m][:, h * CW:(h + 1) * CW],
                                    wt[:, k * CB + m * 128:
                                       k * CB + (m + 1) * 128],
                                    ak[:, h * CW:(h + 1) * CW],
                                    start=(k == 0), stop=(k == KT - 1))
                    for m in range(2):
                        gsb = pool.tile([128, 1024], bf16, tag="gsb", bufs=2)
                        nc.vector.tensor_copy(gsb[:], psg[m][:])
                        for h in range(2):
                            n = 2 * p + h
                            nc.gpsimd.dma_start(
                                gin[n][m * 128:(m + 1) * 128, :],
                                gsb[:, h * CW:(h + 1) * CW])
                    for h in range(2):
                        n = 2 * p + h
                        # triggers are pre-queued back-to-back: nothing
                        # between them waits on a prior AG, so the meshes
                        # chain with ~2us gaps instead of ~30us rendezvous
                        nc.gpsimd.collective_compute(
                            "AllGather", mybir.AluOpType.bypass,
                            replica_groups=RG,
                            ins=[gin[n].opt()], outs=[gout[n].opt()])

            # Warm-bridge: garbage matmuls issued between GEMM1 and GEMM3.
            # They execute during the otherwise-idle AllGather-0 wait and
            # keep the HAM clock gate at full rate so GEMM3 starts warm.
            psw2 = psum.tile([128, 512], f32, tag="pb3", name="psw2")
            for _ in range(24):
                nc.tensor.matmul(psw2[:], wt[:, 0:128], wt[:, 0:512],
                                 start=True, stop=True)

            # query^T resident (bf16), issued after the A-stream so it
            # never delays GEMM1; lands well before GEMM3 needs it
            qt_sb = []
            for t in range(CT):
                qts = pool.tile([128, BB], bf16, tag=f"qt{t}", name=f"qts{t}")
                eng = nc.sync if t % 2 == 0 else nc.scalar
                eng.dma_start(qts[:], qt_d.ap()[t * 128:(t + 1) * 128, :])
                qt_sb.append(qts)

            # gathered-G loads: chunk 0 split across sync+scalar (it gates
            # GEMM3 startup); chunks 1-3 on scalar only, where waiting on
            # AG_n blocks nothing that is needed earlier.  sync still has
            # out-stores after, but gr0's dep (AG_0) resolves ~25us before
            # the first out-store is ready, so no HOL risk there.
            grhs_all = []
            for n in range(NCH):
                grh = []
                for t in range(CT):
                    gr = pool.tile([128, CW], bf16, tag=f"gr{n}", bufs=CT,
                                   name=f"gr{n}_{t}")
                    if n == 0:
                        row0 = (t // 2) * (CB + 1) + (t % 2) * 128
                        eng = (nc.sync, nc.scalar, nc.gpsimd)[t % 3]
                    else:
                        row0 = t * 128
                        eng = nc.scalar
                    eng.dma_start(gr[:], gout[n][row0:row0 + 128, :])
                    grh.append(gr)
                grhs_all.append(grh)
                if n == 0:
                    # partial-fro rows from gout0: scalar queue, after gr0
                    # odds but before gr1 (alpha is needed ~20us later)
                    tr_sb = pool.tile([NCORES, 2], bf16, tag="trsb")
                    for r in range(NCORES):
                        row = r * (CB + 1) + CB
                        nc.scalar.dma_start(tr_sb[r:r + 1, :],
                                            gout0[row:row + 1, 0:2])
                    fro2 = pool.tile([1, 1], f32, tag="fro2")
                    nc.gpsimd.tensor_reduce(fro2[:],
                                            tr_sb[:, 0:2].bitcast(f32),
                                            op=mybir.AluOpType.add,
                                            axis=mybir.AxisListType.C)
                    fro2b = pool.tile([128, 1], f32, tag="fro2b")
                    nc.gpsimd.partition_broadcast(fro2b[:], fro2[:])

            # ---- alpha chain: s = C1*min(exp(ls),clamp)/(fro2+1e-8),
            #      computed as [128,1] elementwise so no late broadcast ----
            with nc.named_scope("alpha"):
                emin = pool.tile([128, 1], f32, tag="emin")
                nc.vector.tensor_scalar_min(emin[:], ex[:], ALPHA_CLAMP)
                den = pool.tile([128, 1], f32, tag="den")
                nc.vector.tensor_scalar_add(den[:], fro2b[:], 1e-8)
                r0 = pool.tile([128, 1], f32, tag="r0")
                nc.vector.reciprocal(r0[:], den[:])
                # one Newton step: r = r0*(2 - den*r0)
                t1 = pool.tile([128, 1], f32, tag="t1")
                nc.vector.tensor_mul(t1[:], den[:], r0[:])
                t2 = pool.tile([128, 1], f32, tag="t2")
                nc.vector.tensor_scalar(t2[:], t1[:], -1.0, 2.0,
                                        op0=mybir.AluOpType.mult,
                                        op1=mybir.AluOpType.add)
                rr = pool.tile([128, 1], f32, tag="rr")
                nc.vector.tensor_mul(rr[:], r0[:], t2[:])
                al = pool.tile([128, 1], f32, tag="al")
                nc.vector.tensor_mul(al[:], emin[:], rr[:])
                c1b = pool.tile([128, 1], f32, tag="c1b")
                nc.vector.tensor_scalar_mul(c1b[:], al[:], C1)

            # ---- GEMM3: out_i[:, cols_n] = Q_i @ G[:, cols_n], scaled ----
            with nc.named_scope("gemm3"):
                for n in range(NCH):
                    grh = grhs_all[n]
                    for m in range(BB // 128):
                        po = psum.tile([128, CW], f32, tag=f"pb{m % 4}",
                                       name=f"po{n}_{m}")
                        for t in range(CT):
                            nc.tensor.matmul(
                                po[:], qt_sb[t][:, m * 128:(m + 1) * 128],
                                grh[t][:], start=(t == 0), stop=(t == CT - 1))
                        osb = pool.tile([128, CW], f32, tag="osb", bufs=4)
                        nc.vector.tensor_scalar_mul(osb[:], po[:], c1b[:])
                        nc.sync.dma_start(out_d.ap()[m * 128:(m + 1) * 128,
                                                     n * CW:(n + 1) * CW],
                                          osb[:])
    nc.compile()
    return nc


def _get_nc():
    if "nc" not in _CACHE:
        _CACHE["nc"] = build_nc()
    return _CACHE["nc"]


def _run(query, memory_mean, ben_israel_log_scale, trace=False, trace_cores=None):
    import ml_dtypes
    from concourse import bass_utils

    _install_ntff_shim()
    nc = _get_nc()

    bf = ml_dtypes.bfloat16
    a_bf = np.ascontiguousarray(np.asarray(memory_mean, dtype=np.float32)
                                .astype(bf))
    q_bf = np.asarray(query, dtype=np.float32).astype(bf)
    ls = np.full((128, 1), np.float32(np.asarray(ben_israel_log_scale)),
                 dtype=np.float32)

    in_maps = []
    for i in range(NCORES):
        w_sw = np.ascontiguousarray(
            a_bf[:, i * CB:(i + 1) * CB].reshape(KT, 128, CB)
            .transpose(1, 0, 2).reshape(128, KT * CB))
        in_maps.append({
            "a": a_bf,
            "w": w_sw,
            "qt": np.ascontiguousarray(q_bf[i * BB:(i + 1) * BB, :].T),
            "ls": ls,
        })
    res = bass_utils.run_bass_kernel_spmd(
        nc, in_maps, core_ids=list(range(NCORES)), trace=trace,
        trace_cores=trace_cores)
    out = np.concatenate([res.results[i]["out"] for i in range(NCORES)], axis=0)
    return out, res


def kernel(query, memory_mean, ben_israel_log_scale):
    out, _ = _run(query, memory_mean, ben_israel_log_scale, trace=False)
    return out


# revision 21
# speedup vs baseline: 1.0941x; 1.0941x over previous
"""Trainium2 Bass kernel for nn_CA3RecurrentMatrix (scatter_memory).

Math: the reference's Ben-Israel-Cohen pseudoinverse iteration collapses
algebraically.  With pinv_0 = alpha*A^T, every iterate is pinv_n = p_n(G) A^T
with G = A^T A (C x C), and on eigenvalues g of G the output polynomial is
u_8(g)*g = 1 - (1 - alpha*g)^256 = 256*alpha*g - C(256,2)*(alpha*g)^2 + ...
Because alpha <= 5e-4/||A||_F^2, alpha*g_max ~ 7e-7, so even the QUADRATIC
term is <1e-4 relative (verified numerically: dropping it gives 5.7e-5 max
rel err).  Hence, to well within the 2e-2 gate:

    out = (256*alpha) * query @ (A^T A)

Distribution over 8 cores, all bf16 compute (validated 4.2e-3 max rel err):
core i holds W_i = A[:, R_i] (bf16) and computes G row-block
G[R_i, :] = W_i^T A in four 512-column chunks; each chunk is AllGathered
(bf16, ~2MB) as soon as it is ready, and the big GEMM out_i = Q_i @ G
consumes gathered column-chunks as they land.  The partial Frobenius sum
sum(W_i^2) rides as one extra payload row in the FIRST AllGather (f32
bit-split into two bf16 slots), so there is no separate AllReduce on the
serial CC ring; the scale s = 256*min(exp(ls),5e-4)/(fro2+1e-8) is applied
during the PSUM->SBUF eviction of the output tiles.  Queue discipline
(each engine DMA queue is strict FIFO, so placement avoids head-of-line
blocking): sync = W + A-stream evens + out stores; scalar = exp(ls) +
A-stream odds + QT + gathered-G loads; gpsimd = gin stores + all four
AllGather triggers pre-queued back-to-back (nothing between them waits on
a prior AG, so the meshes chain with ~2us gaps instead of ~30us
rendezvous stalls).  The first collective mesh cannot begin before a
fixed ~84us firmware-init wall regardless of trigger time, which sets the
floor: ~84 (wall) + ~28 (AG0) + ~135 (tensor-bound GEMM3 at ~259ns per
512-row matmul) ~= 250-275us.
"""
import sys, os, types

sys.path.insert(0, "/opt/trn_rl_repo")

import numpy as np

B, C, K = 8192, 2048, 4096
NCORES = 8
CB = C // NCORES     # 256 G-row block per core
BB = B // NCORES     # 1024 query rows per core
KT = K // 128        # 32 k-tiles over K
CT = C // 128        # 16 c-tiles over C
CW = 512             # column-chunk width
NCH = C // CW        # 4 column chunks
ALPHA_CLAMP = 5e-4
C1 = 256.0

_CACHE = {}


def _install_ntff_shim():
    """Make trace=True work under axon (antenv.axon_hooks is absent here)."""
    if "antenv.axon_hooks" in sys.modules:
        return
    try:
        import antenv
    except ImportError:
        return
    mod = types.ModuleType("antenv.axon_hooks")
    state = {"hook": None, "resolved": False}

    def set_axon_ntff_profile_hook(hook):
        state["hook"], state["resolved"] = hook, True

    def get_axon_ntff_profile_hook():
        if not state["resolved"]:
            state["resolved"] = True
            try:
                if "/root/.axon_site" not in sys.path:
                    sys.path.insert(0, "/root/.axon_site")
                from trn_agent_boot.trn_boot import _ntff_profile_via_ctypes
                state["hook"] = _ntff_profile_via_ctypes("/opt/axon/libaxon_pjrt.so")
            except Exception:
                state["hook"] = None
        return state["hook"]

    mod.set_axon_ntff_profile_hook = set_axon_ntff_profile_hook
    mod.get_axon_ntff_profile_hook = get_axon_ntff_profile_hook
    sys.modules["antenv.axon_hooks"] = mod
    antenv.axon_hooks = mod


def build_nc():
    import concourse.bacc as bacc
    import concourse.mybir as mybir
    from concourse import tile

    f32 = mybir.dt.float32
    bf16 = mybir.dt.bfloat16
    RG = [list(range(NCORES))]

    nc = bacc.Bacc("TRN2", target_bir_lowering=False, debug=False,
                   num_devices=NCORES)
    a_d = nc.dram_tensor("a", (K, C), bf16, kind="ExternalInput")
    # host pre-swizzled: [128, KT*CB] so rows are 16KB contiguous
    w_d = nc.dram_tensor("w", (128, KT * CB), bf16, kind="ExternalInput")
    qt_d = nc.dram_tensor("qt", (C, BB), bf16, kind="ExternalInput")
    ls_d = nc.dram_tensor("ls", (128, 1), f32, kind="ExternalInput")
    out_d = nc.dram_tensor("out", (BB, C), f32, kind="ExternalOutput")

    with tile.TileContext(nc) as tc:
        with tc.tile_pool(name="sbuf", bufs=1) as pool, \
             tc.tile_pool(name="psum", bufs=1, space="PSUM") as psum, \
             tc.tile_pool(name="dram", bufs=1, space="DRAM") as dram:
            # chunk 0 carries one extra payload row per rank: the partial
            # Frobenius sum (f32 bit-split into two bf16 slots)
            gin0 = dram.tile([CB + 1, CW], bf16, name="gin0")
            gout0 = dram.tile([(CB + 1) * NCORES, CW], bf16,
                              addr_space="Shared", name="gout0")
            gin = [gin0] + [dram.tile([CB, CW], bf16, name=f"gin{n}")
                            for n in range(1, NCH)]
            gout = [gout0] + [dram.tile([C, CW], bf16, addr_space="Shared",
                                        name=f"gout{n}")
                              for n in range(1, NCH)]

            ls_sb = pool.tile([128, 1], f32, tag="ls")
            nc.gpsimd.dma_start(ls_sb[:], ls_d.ap()[:, :])
            # exp(ls) early on the scalar queue (before QT) so the alpha
            # chain never waits on scalar-queue HOL
            ex = pool.tile([128, 1], f32, tag="ex")
            nc.scalar.activation(ex[:], ls_sb[:],
                                 mybir.ActivationFunctionType.Exp)

            # W resident: [128, KT*CB], k-tile k at cols [k*CB:(k+1)*CB].
            # 4 DMAs so GEMM1 k-tile 0 isn't gated on the whole 2MB.
            wt = pool.tile([128, KT * CB], bf16, tag="wt")
            for j in range(4):
                sl = slice(j * 8 * CB, (j + 1) * 8 * CB)
                nc.sync.dma_start(wt[:, sl], w_d.ap()[:, sl])

            # PE warm-up: garbage matmuls on the first W quarter to lift the
            # HAM clock gate while the A-stream lands (never read back)
            psw = psum.tile([128, 512], f32, tag="pb3", name="psw")
            for _ in range(8):
                nc.tensor.matmul(psw[:], wt[:, 0:128], wt[:, 0:512],
                                 start=True, stop=True)

            # ---- partial fro2 = sum(W_i^2) on DVE ----
            with nc.named_scope("fro2"):
                dp = pool.tile([128, 4], f32, tag="dp")
                for j in range(4):
                    sl = slice(j * 8 * CB, (j + 1) * 8 * CB)
                    sq = pool.tile([128, 8 * CB], f32, tag="sq", bufs=1)
                    nc.vector.tensor_mul(sq[:], wt[:, sl], wt[:, sl])
                    nc.vector.reduce_sum(dp[:, j:j + 1], sq[:],
                                         axis=mybir.AxisListType.X)
                dx = pool.tile([128, 1], f32, tag="dx")
                nc.vector.reduce_sum(dx[:], dp[:], axis=mybir.AxisListType.X)
                fro_p = pool.tile([1, 1], f32, tag="frop")
                nc.gpsimd.tensor_reduce(fro_p[:], dx[:],
                                        op=mybir.AluOpType.add,
                                        axis=mybir.AxisListType.C)
                # ride along in AllGather chunk 0 (bit-exact f32 in 2 bf16)
                # NB: on gpsimd, not scalar — scalar would HOL-block the
                # A-odd stream behind fro_p's dependency
                nc.gpsimd.dma_start(gin0[CB:CB + 1, 0:2].bitcast(f32),
                                    fro_p[:])

            # ---- GEMM1: two 1024-wide passes over A (2KB DMA rows, split
            #      across sync+scalar); evict per-512 chunk and AllGather
            #      each chunk as soon as its pass completes ----
            with nc.named_scope("gemm1"):
                for p in range(2):
                    psg = [psum.tile([128, 1024], f32, tag=f"pa{m}",
                                     name=f"psg{p}_{m}") for m in range(2)]
                    for k in range(KT):
                        ak = pool.tile([128, 1024], bf16, tag="ak", bufs=8)
                        eng = nc.sync if k % 2 == 0 else nc.scalar
                        eng.dma_start(ak[:],
                                      a_d.ap()[k * 128:(k + 1) * 128,
                                               p * 1024:(p + 1) * 1024])
                        for m in range(2):
                            for h in range(2):
                                nc.tensor.matmul(
                                    psg[m][:, h * CW:(h + 1) * CW],
                                    wt[:, k * CB + m * 128:
                                       k * CB + (m + 1) * 128],
                                    ak[:, h * CW:(h + 1) * CW],
                                    start=(k == 0), stop=(k == KT - 1))
                    for m in range(2):
                        gsb = pool.tile([128, 1024], bf16, tag="gsb", bufs=2)
                        nc.vector.tensor_copy(gsb[:], psg[m][:])
                        for h in range(2):
                            n = 2 * p + h
                            nc.gpsimd.dma_start(
                                gin[n][m * 128:(m + 1) * 128, :],
                                gsb[:, h * CW:(h + 1) * CW])
                    for h in range(2):
                        n = 2 * p + h
                        # triggers are pre-queued back-to-back: nothing
                        # between them waits on a prior AG, so the meshes
                        # chain with ~2us gaps instead of ~30us rendezvous
                        nc.gpsimd.collective_compute(
                            "AllGather", mybir.AluOpType.bypass,
                            replica_groups=RG,
                            ins=[gin[n].opt()], outs=[gout[n].opt()])

            # query^T resident (bf16), issued after the A-stream so it
            # never delays GEMM1; lands well before GEMM3 needs it
            qt_sb = []
            for t in range(CT):
                qts = pool.tile([128, BB], bf16, tag=f"qt{t}", name=f"qts{t}")
                eng = nc.sync if t % 2 == 0 else nc.scalar
                eng.dma_start(qts[:], qt_d.ap()[t * 128:(t + 1) * 128, :])
                qt_sb.append(qts)

            # gathered-G loads: chunk 0 split across sync+scalar (it gates
            # GEMM3 startup); chunks 1-3 on scalar only, where waiting on
            # AG_n blocks nothing that is needed earlier.  sync still has
            # out-stores after, but gr0's dep (AG_0) resolves ~25us before
            # the first out-store is ready, so no HOL risk there.
            grhs_all = []
            for n in range(NCH):
                grh = []
                for t in range(CT):
                    gr = pool.tile([128, CW], bf16, tag=f"gr{n}", bufs=CT,
                                   name=f"gr{n}_{t}")
                    if n == 0:
                        row0 = (t // 2) * (CB + 1) + (t % 2) * 128
                        eng = nc.sync if t % 2 == 0 else nc.scalar
                    else:
                        row0 = t * 128
                        eng = nc.scalar
                    eng.dma_start(gr[:], gout[n][row0:row0 + 128, :])
                    grh.append(gr)
                grhs_all.append(grh)
                if n == 0:
                    # partial-fro rows from gout0: scalar queue, after gr0
                    # odds but before gr1 (alpha is needed ~20us later)
                    tr_sb = pool.tile([NCORES, 2], bf16, tag="trsb")
                    for r in range(NCORES):
                        row = r * (CB + 1) + CB
                        nc.scalar.dma_start(tr_sb[r:r + 1, :],
                                            gout0[row:row + 1, 0:2])
                    fro2 = pool.tile([1, 1], f32, tag="fro2")
                    nc.gpsimd.tensor_reduce(fro2[:],
                                            tr_sb[:, 0:2].bitcast(f32),
                                            op=mybir.AluOpType.add,
                                            axis=mybir.AxisListType.C)
                    fro2b = pool.tile([128, 1], f32, tag="fro2b")
                    nc.gpsimd.partition_broadcast(fro2b[:], fro2[:])

            # ---- alpha chain: s = C1*min(exp(ls),clamp)/(fro2+1e-8),
            #      computed as [128,1] elementwise so no late broadcast ----
            with nc.named_scope("alpha"):
                emin = pool.tile([128, 1], f32, tag="emin")
                nc.vector.tensor_scalar_min(emin[:], ex[:], ALPHA_CLAMP)
                den = pool.tile([128, 1], f32, tag="den")
                nc.vector.tensor_scalar_add(den[:], fro2b[:], 1e-8)
                r0 = pool.tile([128, 1], f32, tag="r0")
                nc.vector.reciprocal(r0[:], den[:])
                # one Newton step: r = r0*(2 - den*r0)
                t1 = pool.tile([128, 1], f32, tag="t1")
                nc.vector.tensor_mul(t1[:], den[:], r0[:])
                t2 = pool.tile([128, 1], f32, tag="t2")
                nc.vector.tensor_scalar(t2[:], t1[:], -1.0, 2.0,
                                        op0=mybir.AluOpType.mult,
                                        op1=mybir.AluOpType.add)
                rr = pool.tile([128, 1], f32, tag="rr")
                nc.vector.tensor_mul(rr[:], r0[:], t2[:])
                al = pool.tile([128, 1], f32, tag="al")
                nc.vector.tensor_mul(al[:], emin[:], rr[:])
                c1b = pool.tile([128, 1], f32, tag="c1b")
                nc.vector.tensor_scalar_mul(c1b[:], al[:], C1)

            # ---- GEMM3: out_i[:, cols_n] = Q_i @ G[:, cols_n], scaled ----
            with nc.named_scope("gemm3"):
                for n in range(NCH):
                    grh = grhs_all[n]
                    for m in range(BB // 128):
                        po = psum.tile([128, CW], f32, tag=f"pb{m % 4}",
                                       name=f"po{n}_{m}")
                        for t in range(CT):
                            nc.tensor.matmul(
                                po[:], qt_sb[t][:, m * 128:(m + 1) * 128],
                                grh[t][:], start=(t == 0), stop=(t == CT - 1))
                        osb = pool.tile([128, CW], f32, tag="osb", bufs=4)
                        nc.vector.tensor_scalar_mul(osb[:], po[:], c1b[:])
                        nc.sync.dma_start(out_d.ap()[m * 128:(m + 1) * 128,
                                                     n * CW:(n + 1) * CW],
                                          osb[:])
    nc.compile()
    return nc


def _get_nc():
    if "nc" not in _CACHE:
        _CACHE["nc"] = build_nc()
    return _CACHE["nc"]


def _run(query, memory_mean, ben_israel_log_scale, trace=False, trace_cores=None):
    import ml_dtypes
    from concourse import bass_utils

    _install_ntff_shim()
    nc = _get_nc()

    bf = ml_dtypes.bfloat16
    a_bf = np.ascontiguousarray(np.asarray(memory_mean, dtype=np.float32)
                                .astype(bf))
    q_bf = np.asarray(query, dtype=np.float32).astype(bf)
    ls = np.full((128, 1), np.float32(np.asarray(ben_israel_log_scale)),
                 dtype=np.float32)

    in_maps = []
    for i in range(NCORES):
        w_sw = np.ascontiguousarray(
            a_bf[:, i * CB:(i + 1) * CB].reshape(KT, 128, CB)
            .transpose(1, 0, 2).reshape(128, KT * CB))
        in_maps.append({
            "a": a_bf,
            "w": w_sw,
            "qt": np.ascontiguousarray(q_bf[i * BB:(i + 1) * BB, :].T),
            "ls": ls,
        })
    res = bass_utils.run_bass_kernel_spmd(
        nc, in_maps, core_ids=list(range(NCORES)), trace=trace,
        trace_cores=trace_cores)
    out = np.concatenate([res.results[i]["out"] for i in range(NCORES)], axis=0)
    return out, res


def kernel(query, memory_mean, ben_israel_log_scale):
    out, _ = _run(query, memory_mean, ben_israel_log_scale, trace=False)
    return out
